# revision 4
# baseline (speedup 1.0000x reference)
"""Trainium2 Bass kernel for InterventionAwareStructure loss.

loss = sum_b,i,d A[b,i,d] * mask[regimes[b], d] / count   (scalar)

Data-parallel over batch across 8 NeuronCores. The 2e-2 relative-error
budget is ~60x wider than fp16 rounding on this dot product, so each
core's A shard is staged in HBM as fp16, halving the DMA stream that
dominates the runtime (memory-bound kernel).

Per core (shard [32, 512, 512] fp16 = 16.8 MB):
  - stream 8 chunks of 2 MB (4 batch items each) via SWDGE DMA,
  - VectorE folds the inner row axis 16 -> 8 with one 16-bit add
    (2x DVE perf mode),
  - TensorE reduces the remaining 8x via a block-ones stationary
    matrix, PSUM accumulating per-batch column sums in fp32,
  - VectorE multiplies by the gathered mask rows; ScalarE (ACT)
    accumulate-reduces the product to one dot per batch item.
  - The final chunk is streamed as two half-tiles so the post-stream
    serial chain (fold + matmul + dot) only covers half a chunk.

Work split keeps every engine under the ~42 us DMA stream: DVE ~24 us,
PE ~27 us, ACT ~5 us.

The tiny gather mask[regimes] (256x512) and the final scalar reduction
are done on host; they are negligible next to the A stream.
"""

import numpy as np

import concourse.bass as bass
import concourse.tile as tile
from concourse import bacc, mybir
from concourse.bass_utils import run_bass_kernel_spmd

INTERVENTION_STRENGTH = 1.0

N_CORES = 8
B, N_REGIMES, D = 256, 16, 512
B_SH = B // N_CORES          # 32 batch items per core
CH = 4                       # batch items per chunk (2 MB fp16 per DMA)
NCHUNKS = B_SH // CH         # 8 chunks
FREE = CH * D * D // 128     # 8192 fp16 per partition per chunk tile
HALF = FREE // 2             # 4096: one DVE fold 16 -> 8 row-groups
NMM = HALF // D              # 8 matmuls of free-dim 512 per chunk
ROWS_PER_PART = 128 // CH    # partitions per batch item in the selector

_CACHED_NC = None


def _build_nc() -> bass.Bass:
    nc = bacc.Bacc()
    f32 = mybir.dt.float32
    f16 = mybir.dt.float16
    ACT_COPY = mybir.ActivationFunctionType.Copy

    a = nc.dram_tensor("a", [B_SH, D, D], f16, kind="ExternalInput")
    m = nc.dram_tensor("m", [B_SH, D], f32, kind="ExternalInput")
    out = nc.dram_tensor("out", [CH, NCHUNKS], f32, kind="ExternalOutput")

    # Block-ones selector: W[p, q] = 1 if p // ROWS_PER_PART == q.
    # out[q, d] = sum_p W[p, q] * X[p, d] -> per-batch column sums.
    wsel_np = np.zeros((128, CH), dtype=np.float16)
    wsel_np[np.arange(128), np.arange(128) // ROWS_PER_PART] = 1.0
    wsel = nc.inline_tensor(wsel_np, "wsel")

    # Chunk g of CH batch items -> SBUF tile [128, FREE]: partition
    # p = (gb * ROWS_PER_PART + ih) holds rows i = ih*16 + il of batch
    # item b = g*CH + gb; free axis = (il, d). Each partition line is
    # one 16 KB contiguous DRAM run (16 rows x 512 x 2B).
    a_view = a.rearrange(
        "(ng gb) (ih il) d -> ng (gb ih) (il d)", ng=NCHUNKS, ih=ROWS_PER_PART
    )
    LAST = NCHUNKS - 1

    with tile.TileContext(nc) as tc:
        with (
            tc.tile_pool(name="big", bufs=NCHUNKS - 1) as big_pool,
            tc.tile_pool(name="last", bufs=1) as last_pool,
            tc.tile_pool(name="t1", bufs=2) as t1_pool,
            tc.tile_pool(name="small", bufs=1) as small_pool,
            tc.tile_pool(name="mask", bufs=4) as mask_pool,
            tc.tile_pool(name="tmp", bufs=4) as tmp_pool,
            tc.tile_pool(name="psum", bufs=4, space="PSUM") as psum_pool,
        ):
            # All chunk buffers live in SBUF simultaneously (16 MB), so
            # the HBM stream never waits on compute. The last chunk is
            # two half-tiles: its fold/matmul work can start when the
            # first half lands, halving the post-stream serial chain.
            a_tiles = []
            for g in range(LAST):
                a_t = big_pool.tile([128, FREE], f16, tag="a")
                nc.gpsimd.dma_start(a_t[:], a_view[g])
                a_tiles.append(a_t)
            a_ha = last_pool.tile([128, HALF], f16, tag="ha")
            nc.gpsimd.dma_start(a_ha[:], a_view[LAST][:, :HALF])
            a_hb = last_pool.tile([128, HALF], f16, tag="hb")
            nc.gpsimd.dma_start(a_hb[:], a_view[LAST][:, HALF:])

            # Small transfers ride the scalar-engine HWDGE ring so they
            # never stall the gpsimd descriptor stream.
            w_t = small_pool.tile([128, CH], f16)
            nc.scalar.dma_start(w_t[:], wsel[:])

            o_t = small_pool.tile([CH, NCHUNKS], f32)

            for g in range(NCHUNKS):
                mask_t = mask_pool.tile([CH, D], f32, tag="mask")
                nc.scalar.dma_start(mask_t[:], m[g * CH:(g + 1) * CH, :])

                ps = psum_pool.tile([CH, D], f32, tag="ps")
                if g < LAST:
                    # One DVE fold (il pairs across tile halves), then
                    # 8 accumulating matmuls.
                    a_t = a_tiles[g]
                    t1 = t1_pool.tile([128, HALF], f16, tag="t1")
                    nc.vector.tensor_add(t1[:], a_t[:, :HALF], a_t[:, HALF:])
                    for j in range(NMM):
                        nc.tensor.matmul(
                            ps[:],
                            w_t[:],
                            t1[:, j * D:(j + 1) * D],
                            start=(j == 0),
                            stop=(j == NMM - 1),
                        )
                else:
                    # Last chunk: fold each half independently (il pairs
                    # within the half), 4 matmuls per half.
                    QU = HALF // 2
                    NH = NMM // 2
                    u1 = last_pool.tile([128, QU], f16, tag="u1")
                    nc.vector.tensor_add(u1[:], a_ha[:, :QU], a_ha[:, QU:])
                    for j in range(NH):
                        nc.tensor.matmul(
                            ps[:],
                            w_t[:],
                            u1[:, j * D:(j + 1) * D],
                            start=(j == 0),
                            stop=False,
                        )
                    u2 = last_pool.tile([128, QU], f16, tag="u2")
                    nc.vector.tensor_add(u2[:], a_hb[:, :QU], a_hb[:, QU:])
                    for j in range(NH):
                        nc.tensor.matmul(
                            ps[:],
                            w_t[:],
                            u2[:, j * D:(j + 1) * D],
                            start=False,
                            stop=(j == NH - 1),
                        )

                tmp = tmp_pool.tile([CH, D], f32, tag="tmp")
                nc.vector.tensor_mul(tmp[:], ps[:], mask_t[:])
                # Free-axis sum on the (otherwise idle) scalar engine.
                act_o = tmp_pool.tile([CH, D], f32, tag="act")
                nc.scalar.activation(
                    act_o[:],
                    tmp[:],
                    ACT_COPY,
                    accum_out=o_t[:, g:g + 1],
                )

            nc.scalar.dma_start(out[:], o_t[:])

    nc.finalize()
    return nc


def _get_nc() -> bass.Bass:
    global _CACHED_NC
    if _CACHED_NC is None:
        _CACHED_NC = _build_nc()
    return _CACHED_NC


def _run(a_shards, m_shards, **run_kwargs):
    nc = _get_nc()
    in_maps = [
        {"a": np.ascontiguousarray(a_shards[c]), "m": np.ascontiguousarray(m_shards[c])}
        for c in range(N_CORES)
    ]
    return run_bass_kernel_spmd(nc, in_maps, list(range(N_CORES)), **run_kwargs)


def kernel(A_per_env, intervention_mask, regimes, _run_kwargs=None):
    A_per_env = np.asarray(A_per_env)
    intervention_mask = np.asarray(intervention_mask, dtype=np.float32)
    regs = np.asarray(regimes).astype(np.int64)

    n_regimes = intervention_mask.shape[0]
    valid = regs < n_regimes
    e = np.clip(regs, 0, n_regimes - 1)
    masks = intervention_mask[e] * valid[:, None].astype(np.float32)  # [B, D]

    a_shards = [
        A_per_env[c * B_SH:(c + 1) * B_SH].astype(np.float16) for c in range(N_CORES)
    ]
    m_shards = [masks[c * B_SH:(c + 1) * B_SH] for c in range(N_CORES)]

    res = _run(a_shards, m_shards, **(_run_kwargs or {}))
    num = np.float64(0.0)
    for c in range(N_CORES):
        num += res.results[c]["out"].astype(np.float64).sum()

    count = masks.astype(np.float64).sum()
    loss = num / count if count > 0 else num
    out = np.asarray(INTERVENTION_STRENGTH * loss, dtype=np.float32)
    if _run_kwargs is not None:
        return out, res
    return out
